# revision 1
# baseline (speedup 1.0000x reference)
"""Trainium2 Bass kernel for nn_Attention_layer (dense_transformer).

Per batch element b (one NeuronCore each, 8 cores):
  k = wk @ x + bk;  q = wq @ x + bq;  v = wv @ x + bv      (1x1x1 conv)
  per (h,w):  scores^T[j,i] = (q_hw^T k_hw) * 1/sqrt(S)    (contract S=128)
              a^T = softmax over i (free axis, skip-max)    via ACT Exp+accum_out
              att[s,j] = v_hw @ a_hw                        (PE transposes for v^T, a)
  out = wo @ att + bo + x                                   (residual via DMA accum)

pos = d*256 + hw (d-major).  SBUF big tiles are tag-chained in one pool so
attention-phase tensors reuse the x-tile slots after the projections finish.
"""

import numpy as np
import ml_dtypes

import concourse.bacc as bacc
import concourse.tile as tile
from concourse import mybir
from concourse.bass_utils import run_bass_kernel_spmd

F32 = mybir.dt.float32
BF16 = mybir.dt.bfloat16
AF = mybir.ActivationFunctionType

B, C, S, D, H, W = 8, 256, 128, 64, 16, 16
HW = H * W            # 256
NPOS = D * HW         # 16384
NCHUNK = NPOS // 512  # 32
SCALE = float(1.0 / np.sqrt(np.float32(S)))

CFG = {
    "resid_dma_accum": False,  # residual add via DMA accumulate (else gpsimd)
    "loop_n": 1,               # on-device repeats of the whole body (timing)
    "trace": False,
}

_CACHE = {}


def _emit(nc, tc, io, ctx):
    xb, xf, wkT, wqT, wvT, woT, bk, bq, bv, bo, ident, boT, ones, out_d = io

    big = ctx.enter_context(tc.tile_pool(name="big", bufs=5))
    med = ctx.enter_context(tc.tile_pool(name="med", bufs=1))
    ring = ctx.enter_context(tc.tile_pool(name="ring", bufs=4))
    oring = ctx.enter_context(tc.tile_pool(name="oring", bufs=3))
    xring = ctx.enter_context(tc.tile_pool(name="xring", bufs=6))
    pool = ctx.enter_context(tc.tile_pool(name="const", bufs=1))
    pp_mm = ctx.enter_context(tc.tile_pool(name="pp_mm", bufs=4, space="PSUM"))
    pp_et = ctx.enter_context(tc.tile_pool(name="pp_et", bufs=2, space="PSUM"))
    pp_tr = ctx.enter_context(tc.tile_pool(name="pp_tr", bufs=2, space="PSUM"))
    pp_at = pp_mm

    # ---- constants ------------------------------------------------------
    id_sb = pool.tile([128, 128], BF16, tag="ident")
    nc.sync.dma_start(id_sb[:], ident[:])
    w_sb = {}
    for nm, t in (("wk", wkT), ("wq", wqT), ("wv", wvT)):
        for h in range(2):
            w_sb[nm, h] = pool.tile([128, 128], BF16, tag=f"w_{nm}{h}", name=f"w_{nm}{h}")
            nc.sync.dma_start(w_sb[nm, h][:], t[h * 128:(h + 1) * 128, :])
    woT_sb = pool.tile([128, 256], BF16, tag="woT")
    nc.sync.dma_start(woT_sb[:], woT[:])
    b_sb = {}
    for nm, t in (("bk", bk), ("bq", bq), ("bv", bv)):
        b_sb[nm] = pool.tile([128, 1], F32, tag=f"b_{nm}", name=f"b_{nm}")
        nc.sync.dma_start(b_sb[nm][:], t[:])
    boT_sb = pool.tile([1, C], BF16, tag="boT")
    nc.sync.dma_start(boT_sb[:], boT[:])
    ones_sb = pool.tile([1, 512], BF16, tag="ones")
    nc.sync.dma_start(ones_sb[:], ones[:])

    loop_cm = tc.For_i(0, CFG["loop_n"], 1) if CFG["loop_n"] > 1 else None
    if loop_cm is not None:
        ctx.enter_context(loop_cm)

    # ---- big tag-chained tiles (creation order fixes slot reuse) --------
    xb_sb = [big.tile([128, NPOS], BF16, tag="big", name=f"xb_sb{h}") for h in range(2)]
    for h in range(2):
        for qt in range(4):
            qs = slice(qt * NPOS // 4, (qt + 1) * NPOS // 4)
            nc.sync.dma_start(xb_sb[h][:, qs], xb[h * 128:(h + 1) * 128, qs])
    k_sb = big.tile([128, NPOS], BF16, tag="big")
    q_sb = big.tile([128, NPOS], BF16, tag="big")
    v_sb = big.tile([128, NPOS], BF16, tag="big")

    # ---- projections (k/q/v chunk-interleaved, evicts alternate engines) -
    for ch in range(NCHUNK):
        sl = slice(ch * 512, (ch + 1) * 512)
        for j, (nm, dst, bias) in enumerate(
                (("wk", k_sb, "bk"), ("wq", q_sb, "bq"), ("wv", v_sb, "bv"))):
            ps = pp_mm.tile([128, 512], F32, tag="mm", name=f"pj{nm}{ch}")
            nc.tensor.matmul(ps[:], w_sb[nm, 0][:], xb_sb[0][:, sl],
                             start=True, stop=False)
            nc.tensor.matmul(ps[:], w_sb[nm, 1][:], xb_sb[1][:, sl],
                             start=False, stop=True)
            if (3 * ch + j) % 2:
                nc.vector.tensor_scalar_add(dst[:, sl], ps[:], b_sb[bias][:])
            else:
                nc.scalar.activation(dst[:, sl], ps[:], AF.Identity,
                                     bias=b_sb[bias][:], scale=1.0)

    def hw_slice(t, hw):
        # cols {d*256 + hw, d in 0..63} of a [128, NPOS] tile -> [128, 1, 64]
        return t[:].rearrange("p (d hw) -> p d hw", hw=HW)[
            :, :, hw:hw + 1].rearrange("p d hw -> p hw d")

    # ---- v^T via PE transpose: [128,64] -> [64,128], 4 hw per psum bank -
    vT_sb = big.tile([128, NPOS], BF16, tag="big")       # reuses xb0 slot
    for g in range(HW // 4):
        r0 = ((4 * g) // 128) * 64
        ps = pp_tr.tile([128, 512], BF16, tag="tr")
        for u in range(4):
            hw = 4 * g + u
            nc.tensor.matmul(ps[r0:r0 + 64, u * 128:(u + 1) * 128],
                             hw_slice(v_sb, hw), id_sb[:], is_transpose=True,
                             start=(u == 0), stop=(u == 3))
        cs = ((4 * g) % 128) * 128
        if g % 2:
            nc.scalar.copy(vT_sb[r0:r0 + 64, cs:cs + 512], ps[r0:r0 + 64, :])
        else:
            nc.vector.tensor_copy(vT_sb[r0:r0 + 64, cs:cs + 512],
                                  ps[r0:r0 + 64, :])

    aTT_sb = med.tile([128, 64 * 128], BF16, tag="aTT")
    att_sb = big.tile([128, NPOS], BF16, tag="big")      # 7th: reuses xb1 slot
    att_view = att_sb[:].rearrange("p (d hw) -> p d hw", hw=HW)

    # ---- out-projection for one hw half (strided pos slices) ------------
    def emit_out_half(half, d0=0, d1=16):
        hw0 = half * 128
        for dch in range(d0, d1):       # 4 d-values per chunk
            for h in range(2):
                ps = pp_mm.tile([128, 512], F32, tag="mm", name=f"o{half}{dch}{h}")
                rhs = att_view[:, 4 * dch:4 * dch + 4, hw0:hw0 + 128]
                nc.tensor.matmul(ps[:], woT_sb[:, h * 128:(h + 1) * 128],
                                 rhs, start=True, stop=False)
                nc.tensor.matmul(ps[:], boT_sb[:, h * 128:(h + 1) * 128],
                                 ones_sb[:], start=False, stop=True)
                xr = xring.tile([128, 512], F32, tag="xr", name=f"xr{half}{dch}{h}")
                xv = xf[h * 128:(h + 1) * 128, :].rearrange(
                    "p (d hw) -> p d hw", hw=HW)[:, 4 * dch:4 * dch + 4,
                                                 hw0:hw0 + 128]
                nc.sync.dma_start(xr[:], xv)
                ot = oring.tile([128, 512], F32, tag="out", name=f"ot{half}{dch}{h}")
                nc.vector.tensor_add(ot[:], ps[:], xr[:])
                ov = out_d[h * 128:(h + 1) * 128, :].rearrange(
                    "p (d hw) -> p d hw", hw=HW)[:, 4 * dch:4 * dch + 4,
                                                 hw0:hw0 + 128]
                nc.sync.dma_start(ov, ot[:])

    # ---- attention, batches of 16 hw pairs (32 hw) ----------------------
    for batch in range(8):
        denom = ring.tile([128, 16], F32, tag="denom")
        rcp = ring.tile([128, 16], F32, tag="rcp")
        aT_sb = ring.tile([128, 16 * 64], BF16, tag="aT")
        for g8 in range(2):                 # 2 groups of 8 pairs
            ps = pp_et.tile([128, 512], F32, tag="eT")
            for i8 in range(8):
                i = g8 * 8 + i8
                p = batch * 16 + i
                for u in range(2):
                    hw = 2 * p + u
                    nc.tensor.matmul(ps[u * 64:u * 64 + 64,
                                        i8 * 64:(i8 + 1) * 64],
                                     hw_slice(q_sb, hw), hw_slice(k_sb, hw),
                                     start=True, stop=True,
                                     skip_group_check=True)
            esl = slice(g8 * 512, (g8 + 1) * 512)
            nc.scalar.activation(aT_sb[:, esl], ps[:], AF.Exp, scale=SCALE)
            nc.vector.reduce_sum(
                out=denom[:, g8 * 8:(g8 + 1) * 8],
                in_=aT_sb[:, esl].rearrange("p (i f) -> p i f", i=8),
                axis=mybir.AxisListType.X)
            nc.vector.reciprocal(rcp[:, g8 * 8:(g8 + 1) * 8],
                                 denom[:, g8 * 8:(g8 + 1) * 8])
        for i in range(16):
            nc.gpsimd.tensor_scalar_mul(aT_sb[:, i * 64:(i + 1) * 64],
                                        aT_sb[:, i * 64:(i + 1) * 64],
                                        rcp[:, i:i + 1])
        # a^T -> a transposes: 4 pairs per psum bank
        for g in range(4):
            p0 = batch * 16 + 4 * g
            r0 = ((2 * p0) // 128) * 64
            ps = pp_tr.tile([128, 512], BF16, tag="tr")
            for u in range(4):
                i = 4 * g + u
                nc.tensor.matmul(ps[r0:r0 + 64, u * 128:(u + 1) * 128],
                                 aT_sb[:, i * 64:(i + 1) * 64], id_sb[:],
                                 is_transpose=True, start=(u == 0), stop=(u == 3))
            pl = (p0 % 64)
            if g % 2:
                nc.scalar.copy(aTT_sb[r0:r0 + 64, pl * 128:pl * 128 + 512],
                               ps[r0:r0 + 64, :])
            else:
                nc.vector.tensor_copy(aTT_sb[r0:r0 + 64, pl * 128:pl * 128 + 512],
                                      ps[r0:r0 + 64, :])
        # att matmuls: 8 hw per psum bank
        for g in range(4):
            hw0 = batch * 32 + 8 * g
            ps = pp_at.tile([128, 512], F32, tag="mm", name=f"at{batch}{g}")
            for u in range(8):
                hw = hw0 + u
                r0 = (hw // 128) * 64
                hl = hw % 128
                pl = (hw // 2) % 64
                nc.tensor.matmul(
                    ps[:, u * 64:(u + 1) * 64],
                    vT_sb[r0:r0 + 64, hl * 128:(hl + 1) * 128],
                    aTT_sb[r0:r0 + 64, pl * 128 + (hw % 2) * 64:
                           pl * 128 + (hw % 2) * 64 + 64],
                    start=(u == 0), stop=(u == 7))
            dst = att_view[:, :, hw0:hw0 + 8].rearrange("p d hw -> p hw d")
            if g % 2:
                nc.scalar.copy(dst, ps[:])
            else:
                nc.vector.tensor_copy(dst, ps[:])
        if batch == 3:
            emit_out_half(0)
        elif batch == 7:
            emit_out_half(1)


def build():
    key = tuple(sorted((k, v) for k, v in CFG.items() if k != "trace"))
    if key in _CACHE:
        return _CACHE[key]
    nc = bacc.Bacc("TRN2", target_bir_lowering=False, debug=False, num_devices=8)
    xb = nc.dram_tensor("xb", [C, NPOS], BF16, kind="ExternalInput")
    xf = nc.dram_tensor("xf", [C, NPOS], F32, kind="ExternalInput")
    wkT = nc.dram_tensor("wkT", [C, S], BF16, kind="ExternalInput")
    wqT = nc.dram_tensor("wqT", [C, S], BF16, kind="ExternalInput")
    wvT = nc.dram_tensor("wvT", [C, S], BF16, kind="ExternalInput")
    woT = nc.dram_tensor("woT", [S, C], BF16, kind="ExternalInput")
    bk = nc.dram_tensor("bk", [S, 1], F32, kind="ExternalInput")
    bq = nc.dram_tensor("bq", [S, 1], F32, kind="ExternalInput")
    bv = nc.dram_tensor("bv", [S, 1], F32, kind="ExternalInput")
    bo = nc.dram_tensor("bo", [C, 1], F32, kind="ExternalInput")
    ident = nc.dram_tensor("ident", [128, 128], BF16, kind="ExternalInput")
    boT = nc.dram_tensor("boT", [1, C], BF16, kind="ExternalInput")
    ones = nc.dram_tensor("ones", [1, 512], BF16, kind="ExternalInput")
    out_d = nc.dram_tensor("out", [C, NPOS], F32, kind="ExternalOutput")
    from contextlib import ExitStack
    with tile.TileContext(nc) as tc, ExitStack() as ctx:
        _emit(nc, tc, (xb, xf, wkT, wqT, wvT, woT, bk, bq, bv, bo, ident, boT, ones, out_d),
              ctx)
    nc.compile()
    _CACHE[key] = nc
    return nc


def make_in_maps(x, wk, bk, wq, bq, wv, bv, wo, bo):
    bf = ml_dtypes.bfloat16
    x = np.ascontiguousarray(np.asarray(x, dtype=np.float32)).reshape(B, C, NPOS)
    com = {
        "wkT": np.ascontiguousarray(np.asarray(wk, np.float32).T).astype(bf),
        "wqT": np.ascontiguousarray(np.asarray(wq, np.float32).T).astype(bf),
        "wvT": np.ascontiguousarray(np.asarray(wv, np.float32).T).astype(bf),
        "woT": np.ascontiguousarray(np.asarray(wo, np.float32).T).astype(bf),
        "bk": np.asarray(bk, np.float32).reshape(S, 1),
        "bq": np.asarray(bq, np.float32).reshape(S, 1),
        "bv": np.asarray(bv, np.float32).reshape(S, 1),
        "bo": np.asarray(bo, np.float32).reshape(C, 1),
        "ident": np.eye(128, dtype=bf),
        "boT": np.asarray(bo, np.float32).reshape(1, C).astype(bf),
        "ones": np.ones((1, 512), dtype=bf),
    }
    return [dict(com, xf=x[b], xb=x[b].astype(bf)) for b in range(B)]


def run(x, wk, bk, wq, bq, wv, bv, wo, bo, **kw):
    nc = build()
    maps = make_in_maps(x, wk, bk, wq, bq, wv, bv, wo, bo)
    res = run_bass_kernel_spmd(nc, maps, core_ids=list(range(B)), **kw)
    out = np.stack([np.asarray(r["out"]) for r in res.results])
    return out.reshape(B, C, D, H, W).astype(np.float32), res


def kernel(x, wk, bk, wq, bq, wv, bv, wo, bo):
    out, _ = run(x, wk, bk, wq, bq, wv, bv, wo, bo)
    return out

